# revision 11
# baseline (speedup 1.0000x reference)
"""MoE gate (DeepSeek-style noaux_tc routing) Trainium2 kernel.

kernel(**inputs) takes the FULL unsharded inputs
  hidden_states [4, 4096, 7168] f32, weight [256, 7168] f32,
  e_score_correction_bias [256] f32
and returns the FULL outputs (topk_idx [16384, 8] int32,
topk_weight [16384, 8] float32), matching the jax reference.

Sharding: data-parallel over the 16384-token axis across 8 NeuronCores
(2048 tokens each); gate weight + bias replicated.

GEMM schemes (per 128-token x 128-h chunk, PE transposes x on the fly):
  m3 : exact 3-term fp32r decomposition
         x@w ~= xr@[wr|we] (one N=512 matmul) + xe@wr (N=256)
       (f32r = fp32 rounded to 11 explicit mantissa bits RNE; residual
       split drops only the ~2^-24-relative xe@we term).
  mdr: main term xr@wrS (f32r, N=256, weights pre-scaled by 2^14) plus
       a single fp8-e5m2 DoubleRow matmul that computes BOTH correction
       terms at once: xr8@(we*2^14)8 + xe8@(wr*2^14)8.  The DoubleRow
       stationary is the (replicated) weight side, loaded once per
       (h-chunk, E-half) and reused across a 4-subtile (512-token)
       group, so LDWEIGHTS amortizes; the correction accumulates
       transposed ([E,T] layout) and is PE-transposed back and merged
       at the shared 2^14 scale, which is folded into the sigmoid's
       scale argument.  Routing error ~1e-2 rel on idx (near-tie
       flips only), well under the 2e-2 gate.
Routing runs on-chip with DVE top-8 (max8/max_index), index-matched
bias gather, sum-normalization * 2.5.
"""
import sys
sys.path.insert(0, "/opt/trn_rl_repo")
import numpy as np
import concourse.bass as bass
import concourse.tile as tile
from concourse import bacc, mybir

F32 = mybir.dt.float32
F32R = mybir.dt.float32r
F8E5 = mybir.dt.float8e5
U32 = mybir.dt.uint32
I32 = mybir.dt.int32
AF = mybir.ActivationFunctionType
ALU = mybir.AluOpType
AX = mybir.AxisListType

H = 7168
E = 256
NG = 8          # expert groups
GS = E // NG    # group size (32)
NCH = H // 128  # 56 h-chunks
NB = NCH // 4   # 14 blocks of 4 chunks
BIG = 1.0e30
SCALE_BITS = 14
S = float(2 ** SCALE_BITS)


def rnd11(a: np.ndarray) -> np.ndarray:
    """Host replica of HW fp32->fp32r rounding: 11 explicit mantissa
    bits, round-to-nearest-even (verified on hardware)."""
    u = np.asarray(a, dtype=np.float32).view(np.uint32).astype(np.uint64)
    shift = np.uint64(12)
    mask = (np.uint64(1) << shift) - np.uint64(1)
    half = np.uint64(1) << np.uint64(11)
    lsb = (u >> shift) & np.uint64(1)
    u2 = (u + half - np.uint64(1) + lsb) & ~mask
    return u2.astype(np.uint32).view(np.float32)


def _e5(a: np.ndarray):
    import ml_dtypes
    return np.asarray(a, dtype=np.float32).astype(ml_dtypes.float8_e5m2)


def _build(t_core: int, mode: str = "mdr", n_devices: int = 8,
           repeat: int = 1):
    assert t_core % 512 == 0
    ntiles = t_core // 128
    ngroups = t_core // 512
    nc = bacc.Bacc("TRN2", target_bir_lowering=False, debug=False,
                   num_devices=n_devices)

    x_d = nc.dram_tensor("x", [t_core, H], F32, kind="ExternalInput")
    bias_d = nc.dram_tensor("bias_b", [128, E], F32, kind="ExternalInput")
    iota_d = nc.dram_tensor("iota_b", [128, E], F32, kind="ExternalInput")
    ident_d = nc.dram_tensor("ident", [128, 128], F32, kind="ExternalInput")
    if mode == "m3":
        w2_d = nc.dram_tensor("w2", [H, 512], F32R, kind="ExternalInput")
    else:
        wrs_d = nc.dram_tensor("wrs", [H, E], F32R, kind="ExternalInput")
        drw_d = nc.dram_tensor("drw", [H, 2, E], F8E5, kind="ExternalInput")
    idx_d = nc.dram_tensor("idx_out", [t_core, 8], I32, kind="ExternalOutput")
    w_d = nc.dram_tensor("w_out", [t_core, 8], F32, kind="ExternalOutput")

    sig_scale = 1.0 if mode == "m3" else 1.0 / S

    with tile.TileContext(nc) as tc:
        with (
            tc.tile_pool(name="const", bufs=1) as constp,
            tc.tile_pool(name="xin", bufs=2) as xin,
            tc.tile_pool(name="xt", bufs=4) as xtp,
            tc.tile_pool(name="route", bufs=2) as rp,
            tc.tile_pool(name="small", bufs=2) as sp,
            tc.tile_pool(name="tps", bufs=4, space="PSUM") as tps,
            tc.tile_pool(name="lps", bufs=2, space="PSUM") as lps,
        ):
            ident = constp.tile([128, 128], F32)
            nc.sync.dma_start(ident[:], ident_d[:])
            # PE warm-up vs HAM clock gate during first x DMA
            warm_tag = "logits" if mode == "m3" else "mains_0"
            warm = lps.tile([128, 512], F32, name="warm", tag=warm_tag,
                            bufs=2 if mode == "m3" else 1)
            for _ in range(24):
                nc.tensor.transpose(warm[:, 0:128], ident[:], ident[:])
            bias_sb = constp.tile([128, E], F32)
            nc.gpsimd.dma_start(bias_sb[:], bias_d[:])
            iota_sb = constp.tile([128, E], F32)
            nc.gpsimd.dma_start(iota_sb[:], iota_d[:])

            # ---- weights: DMA straight into f32r/fp8 SBUF (values are
            # pre-rounded on host), sliced for startup pipelining,
            # issued on the ACT HWDGE ring so x loads on sync are not
            # blocked.  Loads are (re-)emitted inside the repeat loop so
            # benchmark iterations include them, like the baseline. ----
            NQ = 14
            QC = NCH // NQ
            if mode == "m3":
                w_sb = constp.tile([128, NCH, 512], F32R, name="w_sb")
                wview = w2_d[:].rearrange("(c p) e -> p c e", p=128)

                def emit_wload():
                    for q in range(NQ):
                        nc.scalar.dma_start(
                            w_sb[:, q * QC:(q + 1) * QC, :],
                            wview[:, q * QC:(q + 1) * QC, :])
            else:
                w_sb = constp.tile([128, NCH, E], F32R, name="w_sb")
                wview = wrs_d[:].rearrange("(c p) e -> p c e", p=128)
                drw_sb = constp.tile([128, NCH, 2, E], F8E5, name="drw_sb")
                dview = drw_d[:].rearrange("(c p) two e -> p c two e", p=128)

                def emit_wload():
                    for q in range(NQ):
                        nc.scalar.dma_start(
                            w_sb[:, q * QC:(q + 1) * QC, :],
                            wview[:, q * QC:(q + 1) * QC, :])
                        nc.scalar.dma_start(
                            drw_sb[:, q * QC:(q + 1) * QC, :, :],
                            dview[:, q * QC:(q + 1) * QC, :, :])
                # x8 block tiles: [h, plane, chunk, token(512)] fp8,
                # persistent across groups (rewritten)
                x8blk = [constp.tile([128, 2, 4, 512], F8E5,
                                     name=f"x8_{b}", tag=f"x8_{b}")
                        for b in range(NB)]

            HH = H // 2  # x loaded in H-halves to bound SBUF

            # ================= m3 =================
            def emit_gemm_m3(i):
                logits = lps.tile([128, 512], F32, name=f"lg_{i}",
                                  tag="logits")
                pend = {}

                def tc_block(x_t, hb, b):
                    tb = tps.tile([128, 512], F32, name=f"tb_{i}_{b}",
                                  tag="tb")
                    for jj in range(4):
                        nc.tensor.transpose(
                            tb[:, 128 * jj:128 * (jj + 1)],
                            x_t[:, 128 * (4 * (b - 7 * hb) + jj):
                                128 * (4 * (b - 7 * hb) + jj + 1)],
                            ident[:])
                    xr = xtp.tile([128, 512], F32R, tag="xr",
                                  name=f"xr_{i}_{b}")
                    nc.scalar.copy(xr[:], tb[:])
                    xe = xtp.tile([128, 512], F32R, tag="xe",
                                  name=f"xe_{i}_{b}")
                    nc.vector.tensor_tensor(xe[:], tb[:],
                                            xr[:].bitcast(F32),
                                            op=ALU.subtract)
                    return (xr, xe)

                def mm_block(b, bufs):
                    xr, xe = bufs
                    for jj in range(4):
                        j = 4 * b + jj
                        nc.tensor.matmul(
                            logits[:], xr[:, 128 * jj:128 * (jj + 1)],
                            w_sb[:, j, :], start=(j == 0), stop=False,
                            skip_group_check=True)
                        nc.tensor.matmul(
                            logits[:, 0:256],
                            xe[:, 128 * jj:128 * (jj + 1)],
                            w_sb[:, j, 0:256], start=False,
                            stop=(j == NCH - 1), skip_group_check=True)

                PIPE = 3
                for hb in range(2):
                    x_t = xin.tile([128, HH], F32, tag="x_t",
                                   name=f"x_{i}_{hb}")
                    for q in range(2):
                        nc.sync.dma_start(
                            x_t[:, HH // 2 * q:HH // 2 * (q + 1)],
                            x_d[128 * i:128 * (i + 1),
                                HH * hb + HH // 2 * q:
                                HH * hb + HH // 2 * (q + 1)])
                    for b in range(7 * hb, 7 * (hb + 1)):
                        pend[b] = tc_block(x_t, hb, b)
                        if b - PIPE >= 0:
                            mm_block(b - PIPE, pend.pop(b - PIPE))
                for b in range(NB - PIPE, NB):
                    mm_block(b, pend.pop(b))
                return logits

            # ================= mdr =================
            def emit_group_mdr(g):
                """4 subtiles (512 tokens). Returns per-subtile logits
                SBUF tiles (scaled by 2^14)."""
                mains = [lps.tile([128, 512], F32, name=f"mains_{g}_{m}",
                                  tag=f"mains_{m}", bufs=1)
                         for m in range(2)]
                corr = [lps.tile([128, 512], F32, name=f"corr_{g}_{h}",
                                 tag=f"corr_{h}", bufs=1)
                        for h in range(2)]

                for s in range(4):
                    i = 4 * g + s
                    main_ap = mains[s // 2][:, 256 * (s % 2):
                                            256 * (s % 2 + 1)]
                    pend = {}

                    def tc_block(x_t, hb, b, s=s, i=i):
                        tb = tps.tile([128, 512], F32,
                                      name=f"tb_{i}_{b}", tag="tb")
                        for jj in range(4):
                            nc.tensor.transpose(
                                tb[:, 128 * jj:128 * (jj + 1)],
                                x_t[:, 128 * (4 * (b - 7 * hb) + jj):
                                    128 * (4 * (b - 7 * hb) + jj + 1)],
                                ident[:])
                        xr = xtp.tile([128, 512], F32R, tag="xr",
                                      name=f"xr_{i}_{b}")
                        nc.scalar.copy(xr[:], tb[:])
                        # fp8 planes: xr8 (gpsimd), xe8 (vector)
                        dst0 = x8blk[b][:, 0, :, 128 * s:128 * (s + 1)]
                        dst1 = x8blk[b][:, 1, :, 128 * s:128 * (s + 1)]
                        src = xr[:].bitcast(F32).rearrange(
                            "p (c t) -> p c t", c=4)
                        nc.gpsimd.tensor_copy(dst0, src)
                        nc.vector.tensor_tensor(
                            dst1, tb[:].rearrange("p (c t) -> p c t", c=4),
                            src, op=ALU.subtract)
                        return xr

                    def mm_block(b, xr, main_ap=main_ap, s=s):
                        for jj in range(4):
                            j = 4 * b + jj
                            nc.tensor.matmul(
                                main_ap, xr[:, 128 * jj:128 * (jj + 1)],
                                w_sb[:, j, :], start=(j == 0),
                                stop=(j == NCH - 1),
                                skip_group_check=True)
                        if s == 3:
                            for jj in range(4):
                                j = 4 * b + jj
                                for h in range(2):
                                    nc.tensor.matmul(
                                        corr[h][:],
                                        drw_sb[:, j, :,
                                               128 * h:128 * (h + 1)],
                                        x8blk[b][:, :, jj, :],
                                        start=(j == 0),
                                        stop=(j == NCH - 1),
                                        perf_mode=mybir.MatmulPerfMode
                                        .DoubleRow,
                                        skip_group_check=True)

                    PIPE = 3
                    for hb in range(2):
                        x_t = xin.tile([128, HH], F32, tag="x_t",
                                       name=f"x_{i}_{hb}")
                        for q in range(2):
                            nc.sync.dma_start(
                                x_t[:, HH // 2 * q:HH // 2 * (q + 1)],
                                x_d[128 * i:128 * (i + 1),
                                    HH * hb + HH // 2 * q:
                                    HH * hb + HH // 2 * (q + 1)])
                        for b in range(7 * hb, 7 * (hb + 1)):
                            pend[b] = tc_block(x_t, hb, b)
                            if b - PIPE >= 0:
                                mm_block(b - PIPE, pend.pop(b - PIPE))
                    for b in range(NB - PIPE, NB):
                        mm_block(b, pend.pop(b))

                # corr: evacuate, transpose back, merge with mains
                corr_sb = rp.tile([128, 1024], F32, tag="corr_sb",
                                  name=f"corr_sb_{g}")
                nc.scalar.copy(corr_sb[:, 0:512], corr[0][:])
                nc.scalar.copy(corr_sb[:, 512:1024], corr[1][:])
                lgs = []
                for s in range(4):
                    cbt = tps.tile([128, 512], F32, name=f"cbt_{g}_{s}",
                                   tag="tb")
                    nc.tensor.transpose(
                        cbt[:, 0:128],
                        corr_sb[:, 128 * s:128 * (s + 1)], ident[:])
                    nc.tensor.transpose(
                        cbt[:, 128:256],
                        corr_sb[:, 512 + 128 * s:512 + 128 * (s + 1)],
                        ident[:])
                    lg = rp.tile([128, 256], F32, tag="lgs",
                                 name=f"lgs_{g}_{s}", bufs=8)
                    # DVE reads only one PSUM operand per instruction:
                    # evacuate cbt via ACT, then add mains in-place
                    nc.scalar.copy(lg[:], cbt[:, 0:256])
                    nc.vector.tensor_tensor(
                        lg[:], lg[:],
                        mains[s // 2][:, 256 * (s % 2):256 * (s % 2 + 1)],
                        op=ALU.add)
                    lgs.append(lg)
                return lgs

            # ================= routing =================
            def emit_routing(i, logits, from_psum):
                scores = rp.tile([128, E], F32, tag="scores",
                                 name=f"scores_{i}")
                if from_psum:
                    spre = rp.tile([128, E], F32, tag="spre",
                                   name=f"spre_{i}")
                    nc.scalar.copy(spre[:], logits[:, 256:512])
                    nc.vector.tensor_tensor(spre[:], spre[:],
                                            logits[:, 0:256],
                                            op=ALU.add)
                    nc.scalar.activation(scores[:], spre[:], AF.Sigmoid,
                                         scale=sig_scale)
                else:
                    nc.scalar.activation(scores[:], logits[:],
                                         AF.Sigmoid, scale=sig_scale)
                sfc = rp.tile([128, E], F32, tag="sfc", name=f"sfc_{i}")
                nc.gpsimd.tensor_tensor(sfc[:], scores[:], bias_sb[:],
                                        op=ALU.add)

                g8 = sp.tile([128, 64], F32, tag="g8", name=f"g8_{i}")
                for g in range(NG):
                    nc.vector.max(g8[:, 8 * g:8 * g + 8],
                                  sfc[:, GS * g:GS * (g + 1)])
                gsc = sp.tile([128, NG], F32, tag="gsc", name=f"gsc_{i}")
                nc.vector.tensor_reduce(
                    gsc[:],
                    g8[:].rearrange("p (g i) -> p g i", i=8)[:, :, 0:2],
                    axis=AX.X, op=ALU.add)

                gt8 = sp.tile([128, 8], F32, tag="gt8", name=f"gt8_{i}")
                nc.vector.max(gt8[:], gsc[:])
                pen = sp.tile([128, NG], F32, tag="pen", name=f"pen_{i}")
                nc.vector.tensor_scalar(pen[:], gsc[:], gt8[:, 3:4], -BIG,
                                        op0=ALU.is_lt, op1=ALU.mult)

                masked = rp.tile([128, E], F32, tag="masked",
                                 name=f"masked_{i}")
                for g in range(NG):
                    nc.gpsimd.tensor_scalar_add(
                        masked[:, GS * g:GS * (g + 1)],
                        sfc[:, GS * g:GS * (g + 1)], pen[:, g:g + 1])

                m8 = sp.tile([128, 8], F32, tag="m8", name=f"m8_{i}")
                nc.vector.max(m8[:], masked[:])
                i8 = sp.tile([128, 8], U32, tag="i8", name=f"i8_{i}")
                nc.vector.max_index(i8[:], m8[:], masked[:])

                # w_raw[k] = m8[k] - bias[i8[k]] (index-matched gather)
                i8f = sp.tile([128, 8], F32, tag="i8f", name=f"i8f_{i}")
                nc.vector.tensor_copy(i8f[:], i8[:])
                junk = rp.tile([128, E], F32, tag="junk", name=f"junk_{i}")
                biasg = sp.tile([128, 8], F32, tag="biasg",
                                name=f"biasg_{i}")
                for k in range(8):
                    nc.vector.scalar_tensor_tensor(
                        junk[:], iota_sb[:], i8f[:, k:k + 1], bias_sb[:],
                        op0=ALU.is_equal, op1=ALU.mult,
                        accum_out=biasg[:, k:k + 1])

                wraw = sp.tile([128, 8], F32, tag="wraw", name=f"wraw_{i}")
                nc.vector.tensor_tensor(wraw[:], m8[:], biasg[:],
                                        op=ALU.subtract)
                ssum = sp.tile([128, 1], F32, tag="ssum", name=f"ssum_{i}")
                nc.vector.tensor_reduce(ssum[:], wraw[:], axis=AX.X,
                                        op=ALU.add)
                inv = sp.tile([128, 1], F32, tag="inv", name=f"inv_{i}")
                nc.vector.reciprocal(inv[:], ssum[:])
                wout = sp.tile([128, 8], F32, tag="wout", name=f"wout_{i}")
                nc.vector.tensor_scalar(wout[:], wraw[:], inv[:], 2.5,
                                        op0=ALU.mult, op1=ALU.mult)

                nc.sync.dma_start(idx_d[128 * i:128 * (i + 1), :],
                                  i8[:].bitcast(I32))
                nc.sync.dma_start(w_d[128 * i:128 * (i + 1), :], wout[:])

            def emit_all():
                emit_wload()
                if mode == "m3":
                    held = {}
                    for i in range(ntiles):
                        held[i] = emit_gemm_m3(i)
                        if i >= 1:
                            emit_routing(i - 1, held.pop(i - 1), True)
                    emit_routing(ntiles - 1, held.pop(ntiles - 1), True)
                else:
                    held = {}
                    for g in range(ngroups):
                        held[g] = emit_group_mdr(g)
                        if g >= 1:
                            for s, lg in enumerate(held.pop(g - 1)):
                                emit_routing(4 * (g - 1) + s, lg, False)
                    for s, lg in enumerate(held.pop(ngroups - 1)):
                        emit_routing(4 * (ngroups - 1) + s, lg, False)

            if repeat == 1:
                emit_all()
            else:
                with tc.For_i(0, repeat, 1):
                    emit_all()

    nc.compile()
    return nc


def host_prep(weight: np.ndarray, bias: np.ndarray, mode: str):
    wT = np.ascontiguousarray(weight.T).astype(np.float32)  # [H, E]
    wr = rnd11(wT)
    base = {
        "bias_b": np.ascontiguousarray(
            np.broadcast_to(bias[None, :], (128, E))),
        "iota_b": np.ascontiguousarray(
            np.broadcast_to(np.arange(E, dtype=np.float32)[None, :],
                            (128, E))),
        "ident": np.eye(128, dtype=np.float32),
    }
    if mode == "m3":
        we = rnd11(wT - wr)
        base["w2"] = np.ascontiguousarray(
            np.concatenate([wr, we], axis=1))
    else:
        base["wrs"] = np.ascontiguousarray(wr * S)
        drw = np.stack([_e5((wT - wr) * S), _e5(wr * S)], axis=1)
        base["drw"] = np.ascontiguousarray(drw)
    return base


_NC_CACHE = {}
_T_FULL = 16384
_N_CORES = 8
_GEMM = "mdr"


def kernel(hidden_states, weight, e_score_correction_bias):
    from concourse.bass_utils import run_bass_kernel_spmd

    x = np.ascontiguousarray(
        np.asarray(hidden_states, dtype=np.float32).reshape(_T_FULL, H))
    w = np.asarray(weight, dtype=np.float32)
    bias = np.asarray(e_score_correction_bias, dtype=np.float32)
    t_core = _T_FULL // _N_CORES

    if _GEMM not in _NC_CACHE:
        _NC_CACHE[_GEMM] = _build(t_core, mode=_GEMM, n_devices=_N_CORES)
    nc = _NC_CACHE[_GEMM]

    base = host_prep(w, bias, _GEMM)
    maps = []
    for c in range(_N_CORES):
        m = dict(base)
        m["x"] = np.ascontiguousarray(x[c * t_core:(c + 1) * t_core])
        maps.append(m)

    br = run_bass_kernel_spmd(nc, maps, list(range(_N_CORES)))
    idx = np.concatenate(
        [br.results[c]["idx_out"] for c in range(_N_CORES)],
        axis=0).astype(np.int32)
    wout = np.concatenate(
        [br.results[c]["w_out"] for c in range(_N_CORES)],
        axis=0).astype(np.float32)
    return idx, wout
